# revision 36
# baseline (speedup 1.0000x reference)
"""Trainium2 Bass kernel for nn_Attention_7009386627377.

Multi-head attention (16 heads, d=64) over [4, 2048, 1024] hidden states,
sharded across 8 NeuronCores as (batch b = core//2, head-group g = core%2 of
8 heads). Each core computes its disjoint [2048, 512] output slice with no
collectives; the host reassembles [4, 2048, 16, 64].

Per-core pipeline (bf16 compute, fp32 PSUM accumulation):
  hidden -> bf16 -> TensorE-transpose -> hiddenT
  Qt/Kt = W-stationary matmuls (transposed layout), V natural (+bias, mask)
  per head-pair: scoresT (2-head row-packed, K=64), exp on ScalarE from PSUM
  (scale=1/8 folded), ctxT (2-head col-packed, V stationary), row-sums via
  mask-stationary matmuls sharing one accumulation bank.
  Normalize after a DMA-xbar transpose using per-partition reciprocal sums.
"""
import threading

import numpy as np

B = 4
S = 2048
HID = 1024
JC = 512          # per-core qkv columns = 8 heads x 64
D = 64
N_CORES = 8

_LOCK = threading.Lock()
_CACHE = {}


def _build(s=S):
    from contextlib import ExitStack

    from concourse import bacc, mybir
    import concourse.bass as bass
    import concourse.tile as tile
    from concourse.masks import make_identity

    F32 = mybir.dt.float32
    BF16 = mybir.dt.bfloat16
    EXP = mybir.ActivationFunctionType.Exp
    MUL = mybir.AluOpType.mult
    ADD = mybir.AluOpType.add

    nst = s // 128           # s-tiles
    nq = max(1, s // 512)    # 512-wide quarters of s
    qw = s // nq             # quarter width
    nkt = s // 128           # key tiles

    nc = bacc.Bacc("TRN2", target_bir_lowering=False, debug=False,
                   enable_asserts=False)

    hid = nc.dram_tensor("hidden", [s, HID], F32, kind="ExternalInput").ap()
    msk = nc.dram_tensor("mask", [s, 1], F32, kind="ExternalInput").ap()
    wq_d = nc.dram_tensor("wq", [HID, JC], F32, kind="ExternalInput").ap()
    wk_d = nc.dram_tensor("wk", [HID, JC], F32, kind="ExternalInput").ap()
    wv_d = nc.dram_tensor("wv", [HID, JC], F32, kind="ExternalInput").ap()
    bq_d = nc.dram_tensor("bq", [JC, 1], F32, kind="ExternalInput").ap()
    bk_d = nc.dram_tensor("bk", [JC, 1], F32, kind="ExternalInput").ap()
    bv_d = nc.dram_tensor("bv", [1, JC], F32, kind="ExternalInput").ap()
    out_d = nc.dram_tensor("out", [s, JC], F32, kind="ExternalOutput").ap()

    with tile.TileContext(nc) as tc, ExitStack() as ctx:
        P = ctx.enter_context
        persist = P(tc.tile_pool(name="persist", bufs=1))
        dram_pool = P(tc.tile_pool(name="dram", bufs=1, space="DRAM"))
        hstage_pool = P(tc.tile_pool(name="hstage", bufs=4))
        hbf_pool = P(tc.tile_pool(name="hbf", bufs=3))
        wstage_pool = P(tc.tile_pool(name="wstage", bufs=2))
        pt_pool = P(tc.tile_pool(name="pt", bufs=12))
        ctx_sb_pool = P(tc.tile_pool(name="ctxsb", bufs=2))
        sums_sb_pool = P(tc.tile_pool(name="sumssb", bufs=2))
        outt_pool = P(tc.tile_pool(name="outt", bufs=2))
        outf_pool = P(tc.tile_pool(name="outf", bufs=2))
        # PSUM: "big" = [128,1024] slots (scores, hiddenT transposes),
        # "small" = [128,512] slots (everything else). 2*2 + 4*1 = 8 banks.
        ps_big = P(tc.tile_pool(name="psbig", bufs=2, space="PSUM"))
        ps_small = P(tc.tile_pool(name="pssmall", bufs=4, space="PSUM"))

        ident_bf = persist.tile([128, 128], BF16, tag="ident_bf")
        make_identity(nc, ident_bf[:])
        ident_f32 = persist.tile([128, 128], F32, tag="ident_f32")
        make_identity(nc, ident_f32[:])
        ones_row = persist.tile([1, 128], BF16, tag="ones_row")
        nc.vector.memset(ones_row[:], 1.0)

        # hidden s-tile DMAs issue first so the PE pipeline starts early
        h_stage = []
        for t in range(nst):
            hs = hstage_pool.tile([128, HID], F32, tag="hs", name=f"hs{t}")
            nc.sync.dma_start(hs[:], hid[t * 128:(t + 1) * 128, :])
            h_stage.append(hs)

        # mask [s,1] -> [128, nst] (partition = s%128-within-tile)
        mask_sb = persist.tile([128, nst], F32, tag="mask_sb")
        for t in range(nst):
            nc.scalar.dma_start(mask_sb[:, t:t + 1],
                                msk[t * 128:(t + 1) * 128, :])
        mask_bf = persist.tile([128, nst], BF16, tag="mask_bf")
        nc.vector.tensor_copy(mask_bf[:], mask_sb[:])
        # mask column replicated 32-wide per k-tile: stationary operand of the
        # 4-up packed row-sum matmuls (M=32 per head, 4 heads per PSUM bank)
        mask_rep = persist.tile([128, nst * 32], BF16, tag="mask_rep")
        for t in range(nst):
            nc.vector.tensor_copy(mask_rep[:, t * 32:(t + 1) * 32],
                                  mask_bf[:, t:t + 1].to_broadcast([128, 32]))

        # biases: bq/bk as per-partition columns [128, 4]; bv as a row (bf16)
        bq_sb = persist.tile([128, 4], F32, tag="bq_sb")
        bk_sb = persist.tile([128, 4], F32, tag="bk_sb")
        for p in range(4):
            nc.scalar.dma_start(bq_sb[:, p:p + 1],
                                bq_d[p * 128:(p + 1) * 128, :])
            nc.scalar.dma_start(bk_sb[:, p:p + 1],
                                bk_d[p * 128:(p + 1) * 128, :])
        bv_st = wstage_pool.tile([1, JC], F32, tag="bv_st")
        nc.scalar.dma_start(bv_st[:], bv_d[:, :])
        bv_bf = persist.tile([1, JC], BF16, tag="bv_bf")
        nc.vector.tensor_copy(bv_bf[:], bv_st[:])

        # weights -> bf16 SBUF, chunked by 128 h-rows
        w_sb = {}
        for wname, wd in (("wk", wk_d), ("wq", wq_d), ("wv", wv_d)):
            for hc in range(8):
                st_t = wstage_pool.tile([128, JC], F32, tag="wstage")
                nc.scalar.dma_start(st_t[:], wd[hc * 128:(hc + 1) * 128, :])
                wt = persist.tile([128, JC], BF16, tag=f"{wname}{hc}")
                nc.vector.tensor_copy(wt[:], st_t[:])
                w_sb[(wname, hc)] = wt

        hT = [persist.tile([128, s], BF16, tag=f"hT{hc}", name=f"hT{hc}")
              for hc in range(8)]
        qT = [persist.tile([128, s], BF16, tag=f"qT{p}", name=f"qT{p}")
              for p in range(4)]
        kT = [persist.tile([128, s], BF16, tag=f"kT{p}", name=f"kT{p}")
              for p in range(4)]
        v_sb = [persist.tile([128, JC], BF16, tag=f"v{t}", name=f"v{t}")
                for t in range(nst)]
        recip = persist.tile([128, nst * 8], F32, tag="recip")
        scratch = dram_pool.tile([544, s], BF16, tag="scratch")

        zrow = persist.tile([16, 512], BF16, tag="zrow")
        nc.vector.memset(zrow[:], 0.0)
        for g in range(2):
            for zc in range(s // 512):
                nc.gpsimd.dma_start(
                    scratch[272 * g + 264:272 * g + 272,
                            zc * 512:(zc + 1) * 512], zrow[0:8, :])

        def produce_v(st):
            # V for s-tile st (+bias via K=1 matmul, mask fold on the copy);
            # called from inside the attention stream right before ctx needs it
            vp = ps_small.tile([128, JC], F32, tag="ps", name=f"vp{st}")
            for hc in range(8):
                nc.tensor.matmul(vp[:],
                                 lhsT=hT[hc][:, st * 128:(st + 1) * 128],
                                 rhs=w_sb[("wv", hc)][:],
                                 start=(hc == 0), stop=False)
            nc.tensor.matmul(vp[:], lhsT=ones_row[:], rhs=bv_bf[:],
                             start=False, stop=True)
            nc.vector.tensor_scalar(v_sb[st][:], vp[:],
                                    mask_sb[:, st:st + 1], None, MUL)

        def produce_ht(st):
            hb = hbf_pool.tile([128, HID], BF16, tag="hb")
            nc.vector.tensor_copy(hb[:], h_stage[st][:])
            for hc in range(8):
                tp = ps_big.tile([128, 128], BF16, tag="big")
                nc.tensor.transpose(tp[:], hb[:, hc * 128:(hc + 1) * 128],
                                    ident_bf[:])
                nc.vector.tensor_copy(hT[hc][:, st * 128:(st + 1) * 128],
                                      tp[:])

        def project(dst, wname, b_sb, p, sq):
            pp = ps_small.tile([128, qw], F32, tag="ps", name=f"pp{wname}{p}_{sq}")
            for hc in range(8):
                nc.tensor.matmul(
                    pp[:], lhsT=w_sb[(wname, hc)][:, p * 128:(p + 1) * 128],
                    rhs=hT[hc][:, sq * qw:(sq + 1) * qw],
                    start=(hc == 0), stop=(hc == 7))
            nc.vector.tensor_scalar(dst[p][:, sq * qw:(sq + 1) * qw],
                                    pp[:], b_sb[:, p:p + 1], None, ADD)

        def project_begin(dst, wname, b_sb, p, sq):
            pp = ps_small.tile([128, qw], F32, tag="ps",
                               name=f"pph{wname}{p}_{sq}")
            for hc in range(4):
                nc.tensor.matmul(
                    pp[:], lhsT=w_sb[(wname, hc)][:, p * 128:(p + 1) * 128],
                    rhs=hT[hc][:, sq * qw:(sq + 1) * qw],
                    start=(hc == 0), stop=False)
            return (pp, dst, wname, b_sb, p, sq)

        def project_finish(pp, dst, wname, b_sb, p, sq):
            for hc in range(4, 8):
                nc.tensor.matmul(
                    pp[:], lhsT=w_sb[(wname, hc)][:, p * 128:(p + 1) * 128],
                    rhs=hT[hc][:, sq * qw:(sq + 1) * qw],
                    start=False, stop=(hc == 7))
            nc.vector.tensor_scalar(dst[p][:, sq * qw:(sq + 1) * qw],
                                    pp[:], b_sb[:, p:p + 1], None, ADD)

        # ---- production: hiddenT and K per s-quarter; Q for quarter 0 ----
        st_per_q = qw // 128

        def prod_quarter(sq):
            for st4 in range(st_per_q):
                produce_ht(sq * st_per_q + st4)
            for p in range(4):
                project(kT, "wk", bk_sb, p, sq)
                if sq == 0:
                    project(qT, "wq", bq_sb, p, 0)

        # ---- attention (2-pair groups; software-pipelined kt loops) ----
        class Group:
            def __init__(g, q, r):
                g.q, g.r = q, r
                g.qs = slice(q * qw, (q + 1) * qw)
                g.pA, g.pB = 2 * r, 2 * r + 1
                g.ctxA = ps_small.tile([128, qw], F32, tag="ps",
                                       name=f"ctxA{q}_{r}")
                g.ctxB = ps_small.tile([128, qw], F32, tag="ps",
                                       name=f"ctxB{q}_{r}")
                g.sums = ps_small.tile([128, qw], F32, tag="ps",
                                       name=f"sums{q}_{r}")
                g.prev = None

            def ctx_sums(g, kt, ptA, ptB):
                # ctx heads on disjoint partition ranges (per-range
                # accumulation groups); sums 4-up packed in one bank.
                for ppp, ctx_ps, pt in ((g.pA, g.ctxA, ptA),
                                        (g.pB, g.ctxB, ptB)):
                    nc.tensor.matmul(
                        ctx_ps[0:64, :],
                        lhsT=v_sb[kt][:, ppp * 128:ppp * 128 + 64],
                        rhs=pt[:, 0:qw], start=(kt == 0),
                        stop=(kt == nkt - 1), skip_group_check=True)
                    nc.tensor.matmul(
                        ctx_ps[64:128, :],
                        lhsT=v_sb[kt][:, ppp * 128 + 64:ppp * 128 + 128],
                        rhs=pt[:, qw:2 * qw], start=(kt == 0),
                        stop=(kt == nkt - 1), skip_group_check=True)
                mrep = mask_rep[:, kt * 32:(kt + 1) * 32]
                for i, pt_half in enumerate(
                        (ptA[:, 0:qw], ptA[:, qw:2 * qw],
                         ptB[:, 0:qw], ptB[:, qw:2 * qw])):
                    nc.tensor.matmul(
                        g.sums[32 * i:32 * (i + 1), :], lhsT=mrep,
                        rhs=pt_half, start=(kt == 0),
                        stop=(kt == nkt - 1), skip_group_check=True,
                        tile_position=(0, 32 * i))

            def scores_exp(g, kt):
                ks = slice(kt * 128, (kt + 1) * 128)
                pts = []
                for ppp in (g.pA, g.pB):
                    sc = ps_big.tile([128, 2 * qw], F32, tag="big")
                    nc.tensor.matmul(sc[:, 0:qw], lhsT=kT[ppp][0:64, ks],
                                     rhs=qT[ppp][0:64, g.qs],
                                     start=True, stop=True)
                    nc.tensor.matmul(sc[:, qw:2 * qw],
                                     lhsT=kT[ppp][64:128, ks],
                                     rhs=qT[ppp][64:128, g.qs],
                                     start=True, stop=True)
                    pt = pt_pool.tile([128, 2 * qw], BF16, tag="pt")
                    nc.scalar.activation(pt[:], sc[:], EXP, scale=0.125)
                    pts.append(pt)
                return pts

            def step(g, kt):
                # lag-1 software pipeline: scores/exp of kt precede
                # ctx/sums of kt-1 in the in-order engine streams
                pts = g.scores_exp(kt)
                if g.prev is not None:
                    g.ctx_sums(*g.prev)
                g.prev = (kt, pts[0], pts[1])

            def close(g):
                g.ctx_sums(*g.prev)
                q, r, qs = g.q, g.r, g.qs
                base = 272 * r
                for gi, ctx_ps in ((0, g.ctxA), (1, g.ctxB)):
                    ctx_sb = ctx_sb_pool.tile([128, qw], BF16, tag="ctxsb")
                    nc.vector.tensor_copy(ctx_sb[:], ctx_ps[:])
                    nc.sync.dma_start(
                        scratch[base + gi * 128:base + (gi + 1) * 128, qs],
                        ctx_sb[:])
                # sums rows (partitions 0/32/64/96 = the group's 4 heads)
                # ride in scratch; the per-group xbar transposes them too
                ssb = sums_sb_pool.tile([128, qw], BF16, tag="sumssb")
                for i in range(4):
                    nc.vector.tensor_copy(ssb[32 * i:32 * i + 1, :],
                                          g.sums[32 * i:32 * i + 1, :])
                    nc.sync.dma_start(
                        scratch[base + 256 + i:base + 257 + i, qs],
                        ssb[32 * i:32 * i + 1, :])
                for b4 in range(qw // 128):
                    sbg = q * (qw // 128) + b4
                    ot = outt_pool.tile([128, 272], BF16, tag="outt")
                    nc.sync.dma_start_transpose(
                        ot[:], scratch[base:base + 272,
                                       sbg * 128:(sbg + 1) * 128])
                    rc = persist.tile([128, 4], F32, tag=f"rc{sbg}_{r}",
                                      name=f"rc{sbg}_{r}")
                    nc.vector.reciprocal(rc[:], ot[:, 256:260])
                    of = outf_pool.tile([128, 256], F32, tag="outf")
                    for h in range(4):
                        nc.vector.tensor_scalar(
                            of[:, h * D:(h + 1) * D],
                            ot[:, h * D:(h + 1) * D],
                            rc[:, h:h + 1], None, MUL)
                    nc.sync.dma_start(
                        out_d[sbg * 128:(sbg + 1) * 128,
                              r * 256:(r + 1) * 256], of[:])

        # (q0, r0): a few early scores+exp run inside the production window
        # (they need only big-pool PSUM slots, which production barely uses);
        # their pt tiles park in SBUF and ctx/sums catch up at attention
        # start while the later exps stream on.
        g00 = Group(0, 0)
        dp = {}
        prod_quarter(0)
        for sq in range(1, nq):
            for kt in range(2 * (sq - 1), 2 * sq):
                dp[kt] = g00.scores_exp(kt)
            prod_quarter(sq)
        prev = None
        for kt in range(nkt):
            produce_v(kt)
            pts = dp.pop(kt) if kt in dp else g00.scores_exp(kt)
            if prev is not None:
                g00.ctx_sums(*prev)
            prev = (kt, pts[0], pts[1])
        g00.prev = prev
        g00.close()

        pend = None
        for q in range(nq):
            for r in range(2):
                if q == 0 and r == 0:
                    continue
                g = Group(q, r)
                for kt in range(nkt):
                    if r == 1 and q + 1 < nq and kt in (3, 6, 9, 12):
                        project(qT, "wq", bq_sb, kt // 3 - 1, q + 1)
                    g.step(kt)
                g.close()

    nc.compile()
    return nc


def _get_nc(s=S):
    with _LOCK:
        if s not in _CACHE:
            _CACHE[s] = _build(s)
        return _CACHE[s]


def _make_in_maps(inputs):
    hidden_states = np.asarray(inputs["hidden_states"], dtype=np.float32)
    attention_mask = np.asarray(inputs["attention_mask"], dtype=np.float32)
    Wq = np.asarray(inputs["Wq"], dtype=np.float32)
    Wk = np.asarray(inputs["Wk"], dtype=np.float32)
    Wv = np.asarray(inputs["Wv"], dtype=np.float32)
    bq = np.asarray(inputs["bq"], dtype=np.float32)
    bk = np.asarray(inputs["bk"], dtype=np.float32)
    bv = np.asarray(inputs["bv"], dtype=np.float32)

    in_maps = []
    for core in range(N_CORES):
        b, g = core // 2, core % 2
        js = slice(g * JC, (g + 1) * JC)
        in_maps.append({
            "hidden": np.ascontiguousarray(hidden_states[b]),
            "mask": np.ascontiguousarray(attention_mask[b].reshape(S, 1)),
            "wq": np.ascontiguousarray(Wq[:, js]),
            "wk": np.ascontiguousarray(Wk[:, js]),
            "wv": np.ascontiguousarray(Wv[:, js]),
            "bq": np.ascontiguousarray(bq[js].reshape(JC, 1)),
            "bk": np.ascontiguousarray(bk[js].reshape(JC, 1)),
            "bv": np.ascontiguousarray(bv[js].reshape(1, JC)),
        })
    return in_maps


def kernel(hidden_states, attention_mask, Wq, bq, Wk, bk, Wv, bv):
    from concourse.bass_utils import run_bass_kernel_spmd

    nc = _get_nc()
    in_maps = _make_in_maps(dict(
        hidden_states=hidden_states, attention_mask=attention_mask,
        Wq=Wq, bq=bq, Wk=Wk, bk=bk, Wv=Wv, bv=bv))

    res = run_bass_kernel_spmd(nc, in_maps, core_ids=list(range(N_CORES)))
    out = np.empty((B, S, 16, D), dtype=np.float32)
    for core in range(N_CORES):
        b, g = core // 2, core % 2
        out[b, :, g * 8:(g + 1) * 8, :] = \
            res.results[core]["out"].reshape(S, 8, D)
    return out


# revision 38
# speedup vs baseline: 1.0474x; 1.0474x over previous
"""Trainium2 Bass kernel for nn_Attention_7009386627377.

Multi-head attention (16 heads, d=64) over [4, 2048, 1024] hidden states,
sharded across 8 NeuronCores as (batch b = core//2, head-group g = core%2 of
8 heads). Each core computes its disjoint [2048, 512] output slice with no
collectives; the host reassembles [4, 2048, 16, 64].

Per-core pipeline (bf16 compute, fp32 PSUM accumulation):
  hidden -> bf16 -> TensorE-transpose -> hiddenT
  Qt/Kt = W-stationary matmuls (transposed layout), V natural (+bias, mask)
  per head-pair: scoresT (2-head row-packed, K=64), exp on ScalarE from PSUM
  (scale=1/8 folded), ctxT (2-head col-packed, V stationary), row-sums via
  mask-stationary matmuls sharing one accumulation bank.
  Normalize after a DMA-xbar transpose using per-partition reciprocal sums.
"""
import threading

import numpy as np

B = 4
S = 2048
HID = 1024
JC = 512          # per-core qkv columns = 8 heads x 64
D = 64
N_CORES = 8

_LOCK = threading.Lock()
_CACHE = {}


def _build(s=S):
    from contextlib import ExitStack

    from concourse import bacc, mybir
    import concourse.bass as bass
    import concourse.tile as tile
    from concourse.masks import make_identity

    F32 = mybir.dt.float32
    BF16 = mybir.dt.bfloat16
    EXP = mybir.ActivationFunctionType.Exp
    MUL = mybir.AluOpType.mult
    ADD = mybir.AluOpType.add

    nst = s // 128           # s-tiles
    nq = max(1, s // 512)    # 512-wide quarters of s
    qw = s // nq             # quarter width
    nkt = s // 128           # key tiles

    nc = bacc.Bacc("TRN2", target_bir_lowering=False, debug=False,
                   enable_asserts=False)

    hid = nc.dram_tensor("hidden", [s, HID], F32, kind="ExternalInput").ap()
    msk = nc.dram_tensor("mask", [s, 1], F32, kind="ExternalInput").ap()
    wq_d = nc.dram_tensor("wq", [HID, JC], F32, kind="ExternalInput").ap()
    wk_d = nc.dram_tensor("wk", [HID, JC], F32, kind="ExternalInput").ap()
    wv_d = nc.dram_tensor("wv", [HID, JC], F32, kind="ExternalInput").ap()
    bq_d = nc.dram_tensor("bq", [JC, 1], F32, kind="ExternalInput").ap()
    bk_d = nc.dram_tensor("bk", [JC, 1], F32, kind="ExternalInput").ap()
    bv_d = nc.dram_tensor("bv", [1, JC], F32, kind="ExternalInput").ap()
    out_d = nc.dram_tensor("out", [s, JC], F32, kind="ExternalOutput").ap()

    with tile.TileContext(nc) as tc, ExitStack() as ctx:
        P = ctx.enter_context
        persist = P(tc.tile_pool(name="persist", bufs=1))
        dram_pool = P(tc.tile_pool(name="dram", bufs=1, space="DRAM"))
        hstage_pool = P(tc.tile_pool(name="hstage", bufs=4))
        hbf_pool = P(tc.tile_pool(name="hbf", bufs=6))
        wstage_pool = P(tc.tile_pool(name="wstage", bufs=2))
        pt_pool = P(tc.tile_pool(name="pt", bufs=8))
        ctx_sb_pool = P(tc.tile_pool(name="ctxsb", bufs=2))
        sums_sb_pool = P(tc.tile_pool(name="sumssb", bufs=2))
        outt_pool = P(tc.tile_pool(name="outt", bufs=2))
        outf_pool = P(tc.tile_pool(name="outf", bufs=2))
        # PSUM: "big" = [128,1024] slots (scores, hiddenT transposes),
        # "small" = [128,512] slots (everything else). 2*2 + 4*1 = 8 banks.
        ps_big = P(tc.tile_pool(name="psbig", bufs=2, space="PSUM"))
        ps_small = P(tc.tile_pool(name="pssmall", bufs=4, space="PSUM"))

        ident_bf = persist.tile([128, 128], BF16, tag="ident_bf")
        make_identity(nc, ident_bf[:])
        ident_f32 = persist.tile([128, 128], F32, tag="ident_f32")
        make_identity(nc, ident_f32[:])
        ones_row = persist.tile([1, 128], BF16, tag="ones_row")
        nc.vector.memset(ones_row[:], 1.0)
        # preload the exp table set during the production window
        act_warm = persist.tile([1, 128], F32, tag="act_warm")
        nc.scalar.activation(act_warm[:], ones_row[:], EXP, scale=1.0)

        # hidden s-tile DMAs issue first so the PE pipeline starts early
        h_stage = []
        for t in range(nst):
            hs = hstage_pool.tile([128, HID], F32, tag="hs", name=f"hs{t}")
            nc.sync.dma_start(hs[:], hid[t * 128:(t + 1) * 128, :])
            h_stage.append(hs)

        # mask [s,1] -> [128, nst] (partition = s%128-within-tile)
        mask_sb = persist.tile([128, nst], F32, tag="mask_sb")
        for t in range(nst):
            nc.scalar.dma_start(mask_sb[:, t:t + 1],
                                msk[t * 128:(t + 1) * 128, :])
        mask_bf = persist.tile([128, nst], BF16, tag="mask_bf")
        nc.vector.tensor_copy(mask_bf[:], mask_sb[:])
        # mask column replicated 32-wide per k-tile: stationary operand of the
        # 4-up packed row-sum matmuls (M=32 per head, 4 heads per PSUM bank)
        mask_rep = persist.tile([128, nst * 32], BF16, tag="mask_rep")
        for t in range(nst):
            nc.vector.tensor_copy(mask_rep[:, t * 32:(t + 1) * 32],
                                  mask_bf[:, t:t + 1].to_broadcast([128, 32]))

        # biases: bq/bk as per-partition columns [128, 4]; bv as a row (bf16)
        bq_sb = persist.tile([128, 4], F32, tag="bq_sb")
        bk_sb = persist.tile([128, 4], F32, tag="bk_sb")
        for p in range(4):
            nc.scalar.dma_start(bq_sb[:, p:p + 1],
                                bq_d[p * 128:(p + 1) * 128, :])
            nc.scalar.dma_start(bk_sb[:, p:p + 1],
                                bk_d[p * 128:(p + 1) * 128, :])
        bv_st = wstage_pool.tile([1, JC], F32, tag="bv_st")
        nc.scalar.dma_start(bv_st[:], bv_d[:, :])
        bv_bf = persist.tile([1, JC], BF16, tag="bv_bf")
        nc.vector.tensor_copy(bv_bf[:], bv_st[:])

        # first s-quarter's hidden casts go ahead of the W casts in the
        # DVE queue so the transpose pipeline starts immediately
        hb_pre = {}
        for st in range(min(4, nst)):
            hbp = hbf_pool.tile([128, HID], BF16, tag="hb", name=f"hbpre{st}")
            nc.vector.tensor_copy(hbp[:], h_stage[st][:])
            hb_pre[st] = hbp

        # weights -> bf16 SBUF, chunked by 128 h-rows
        w_sb = {}
        for wname, wd in (("wk", wk_d), ("wq", wq_d), ("wv", wv_d)):
            for hc in range(8):
                st_t = wstage_pool.tile([128, JC], F32, tag="wstage")
                nc.scalar.dma_start(st_t[:], wd[hc * 128:(hc + 1) * 128, :])
                wt = persist.tile([128, JC], BF16, tag=f"{wname}{hc}")
                nc.vector.tensor_copy(wt[:], st_t[:])
                w_sb[(wname, hc)] = wt

        hT = [persist.tile([128, s], BF16, tag=f"hT{hc}", name=f"hT{hc}")
              for hc in range(8)]
        qT = [persist.tile([128, s], BF16, tag=f"qT{p}", name=f"qT{p}")
              for p in range(4)]
        kT = [persist.tile([128, s], BF16, tag=f"kT{p}", name=f"kT{p}")
              for p in range(4)]
        v_sb = [persist.tile([128, JC], BF16, tag=f"v{t}", name=f"v{t}")
                for t in range(nst)]
        recip = persist.tile([128, nst * 8], F32, tag="recip")
        scratch = dram_pool.tile([544, s], BF16, tag="scratch")

        zrow = persist.tile([16, 512], BF16, tag="zrow")
        nc.vector.memset(zrow[:], 0.0)
        for g in range(2):
            for zc in range(s // 512):
                nc.gpsimd.dma_start(
                    scratch[272 * g + 264:272 * g + 272,
                            zc * 512:(zc + 1) * 512], zrow[0:8, :])

        def produce_v(st):
            # V for s-tile st (+bias via K=1 matmul, mask fold on the copy);
            # called from inside the attention stream right before ctx needs it
            vp = ps_small.tile([128, JC], F32, tag="ps", name=f"vp{st}")
            for hc in range(8):
                nc.tensor.matmul(vp[:],
                                 lhsT=hT[hc][:, st * 128:(st + 1) * 128],
                                 rhs=w_sb[("wv", hc)][:],
                                 start=(hc == 0), stop=False)
            nc.tensor.matmul(vp[:], lhsT=ones_row[:], rhs=bv_bf[:],
                             start=False, stop=True)
            nc.vector.tensor_scalar(v_sb[st][:], vp[:],
                                    mask_sb[:, st:st + 1], None, MUL)

        def produce_ht(st):
            hb = hb_pre.pop(st, None)
            if hb is None:
                hb = hbf_pool.tile([128, HID], BF16, tag="hb")
                nc.vector.tensor_copy(hb[:], h_stage[st][:])
            for hc in range(8):
                tp = ps_big.tile([128, 128], BF16, tag="big")
                nc.tensor.transpose(tp[:], hb[:, hc * 128:(hc + 1) * 128],
                                    ident_bf[:])
                nc.vector.tensor_copy(hT[hc][:, st * 128:(st + 1) * 128],
                                      tp[:])

        def project(dst, wname, b_sb, p, sq):
            pp = ps_small.tile([128, qw], F32, tag="ps", name=f"pp{wname}{p}_{sq}")
            for hc in range(8):
                nc.tensor.matmul(
                    pp[:], lhsT=w_sb[(wname, hc)][:, p * 128:(p + 1) * 128],
                    rhs=hT[hc][:, sq * qw:(sq + 1) * qw],
                    start=(hc == 0), stop=(hc == 7))
            nc.vector.tensor_scalar(dst[p][:, sq * qw:(sq + 1) * qw],
                                    pp[:], b_sb[:, p:p + 1], None, ADD)

        def project_begin(dst, wname, b_sb, p, sq):
            pp = ps_small.tile([128, qw], F32, tag="ps",
                               name=f"pph{wname}{p}_{sq}")
            for hc in range(4):
                nc.tensor.matmul(
                    pp[:], lhsT=w_sb[(wname, hc)][:, p * 128:(p + 1) * 128],
                    rhs=hT[hc][:, sq * qw:(sq + 1) * qw],
                    start=(hc == 0), stop=False)
            return (pp, dst, wname, b_sb, p, sq)

        def project_finish(pp, dst, wname, b_sb, p, sq):
            for hc in range(4, 8):
                nc.tensor.matmul(
                    pp[:], lhsT=w_sb[(wname, hc)][:, p * 128:(p + 1) * 128],
                    rhs=hT[hc][:, sq * qw:(sq + 1) * qw],
                    start=False, stop=(hc == 7))
            nc.vector.tensor_scalar(dst[p][:, sq * qw:(sq + 1) * qw],
                                    pp[:], b_sb[:, p:p + 1], None, ADD)

        # ---- production: hiddenT and K per s-quarter; Q for quarter 0 ----
        st_per_q = qw // 128

        def prod_quarter(sq):
            for st4 in range(st_per_q):
                produce_ht(sq * st_per_q + st4)
            for p in range(4):
                project(kT, "wk", bk_sb, p, sq)
                if sq == 0:
                    project(qT, "wq", bq_sb, p, 0)

        # ---- attention (2-pair groups; software-pipelined kt loops) ----
        class Group:
            def __init__(g, q, r):
                g.q, g.r = q, r
                g.qs = slice(q * qw, (q + 1) * qw)
                g.pA, g.pB = 2 * r, 2 * r + 1
                g.ctxA = ps_small.tile([128, qw], F32, tag="ps",
                                       name=f"ctxA{q}_{r}")
                g.ctxB = ps_small.tile([128, qw], F32, tag="ps",
                                       name=f"ctxB{q}_{r}")
                g.sums = ps_small.tile([128, qw], F32, tag="ps",
                                       name=f"sums{q}_{r}")
                g.prev = None

            def ctx_sums(g, kt, ptA, ptB):
                # ctx heads on disjoint partition ranges (per-range
                # accumulation groups); sums 4-up packed in one bank.
                for ppp, ctx_ps, pt in ((g.pA, g.ctxA, ptA),
                                        (g.pB, g.ctxB, ptB)):
                    nc.tensor.matmul(
                        ctx_ps[0:64, :],
                        lhsT=v_sb[kt][:, ppp * 128:ppp * 128 + 64],
                        rhs=pt[:, 0:qw], start=(kt == 0),
                        stop=(kt == nkt - 1), skip_group_check=True)
                    nc.tensor.matmul(
                        ctx_ps[64:128, :],
                        lhsT=v_sb[kt][:, ppp * 128 + 64:ppp * 128 + 128],
                        rhs=pt[:, qw:2 * qw], start=(kt == 0),
                        stop=(kt == nkt - 1), skip_group_check=True)
                mrep = mask_rep[:, kt * 32:(kt + 1) * 32]
                for i, pt_half in enumerate(
                        (ptA[:, 0:qw], ptA[:, qw:2 * qw],
                         ptB[:, 0:qw], ptB[:, qw:2 * qw])):
                    nc.tensor.matmul(
                        g.sums[32 * i:32 * (i + 1), :], lhsT=mrep,
                        rhs=pt_half, start=(kt == 0),
                        stop=(kt == nkt - 1), skip_group_check=True,
                        tile_position=(0, 32 * i))

            def scores_exp(g, kt):
                ks = slice(kt * 128, (kt + 1) * 128)
                pts = []
                for ppp in (g.pA, g.pB):
                    sc = ps_big.tile([128, 2 * qw], F32, tag="big")
                    nc.tensor.matmul(sc[:, 0:qw], lhsT=kT[ppp][0:64, ks],
                                     rhs=qT[ppp][0:64, g.qs],
                                     start=True, stop=True)
                    nc.tensor.matmul(sc[:, qw:2 * qw],
                                     lhsT=kT[ppp][64:128, ks],
                                     rhs=qT[ppp][64:128, g.qs],
                                     start=True, stop=True)
                    pt = pt_pool.tile([128, 2 * qw], BF16, tag="pt")
                    nc.scalar.activation(pt[:], sc[:], EXP, scale=0.125)
                    pts.append(pt)
                return pts

            def step(g, kt):
                # lag-1 software pipeline: scores/exp of kt precede
                # ctx/sums of kt-1 in the in-order engine streams
                pts = g.scores_exp(kt)
                if g.prev is not None:
                    g.ctx_sums(*g.prev)
                g.prev = (kt, pts[0], pts[1])

            def close(g):
                g.ctx_sums(*g.prev)
                q, r, qs = g.q, g.r, g.qs
                base = 272 * r
                for gi, ctx_ps in ((0, g.ctxA), (1, g.ctxB)):
                    ctx_sb = ctx_sb_pool.tile([128, qw], BF16, tag="ctxsb")
                    nc.vector.tensor_copy(ctx_sb[:], ctx_ps[:])
                    nc.sync.dma_start(
                        scratch[base + gi * 128:base + (gi + 1) * 128, qs],
                        ctx_sb[:])
                # sums rows (partitions 0/32/64/96 = the group's 4 heads)
                # ride in scratch; the per-group xbar transposes them too
                ssb = sums_sb_pool.tile([128, qw], BF16, tag="sumssb")
                for i in range(4):
                    nc.vector.tensor_copy(ssb[32 * i:32 * i + 1, :],
                                          g.sums[32 * i:32 * i + 1, :])
                    nc.sync.dma_start(
                        scratch[base + 256 + i:base + 257 + i, qs],
                        ssb[32 * i:32 * i + 1, :])
                for b4 in range(qw // 128):
                    sbg = q * (qw // 128) + b4
                    ot = outt_pool.tile([128, 272], BF16, tag="outt")
                    nc.sync.dma_start_transpose(
                        ot[:], scratch[base:base + 272,
                                       sbg * 128:(sbg + 1) * 128])
                    rc = persist.tile([128, 4], F32, tag=f"rc{sbg}_{r}",
                                      name=f"rc{sbg}_{r}")
                    nc.vector.reciprocal(rc[:], ot[:, 256:260])
                    of = outf_pool.tile([128, 256], F32, tag="outf")
                    for h in range(4):
                        nc.vector.tensor_scalar(
                            of[:, h * D:(h + 1) * D],
                            ot[:, h * D:(h + 1) * D],
                            rc[:, h:h + 1], None, MUL)
                    nc.sync.dma_start(
                        out_d[sbg * 128:(sbg + 1) * 128,
                              r * 256:(r + 1) * 256], of[:])

        for sq in range(nq):
            prod_quarter(sq)
        g00 = Group(0, 0)
        for kt in range(nkt):
            produce_v(kt)
            g00.step(kt)
        g00.close()

        pend = None
        for q in range(nq):
            for r in range(2):
                if q == 0 and r == 0:
                    continue
                g = Group(q, r)
                for kt in range(nkt):
                    if r == 1 and q + 1 < nq and kt in (3, 6, 9, 12):
                        project(qT, "wq", bq_sb, kt // 3 - 1, q + 1)
                    g.step(kt)
                g.close()

    nc.compile()
    return nc


def _get_nc(s=S):
    with _LOCK:
        if s not in _CACHE:
            _CACHE[s] = _build(s)
        return _CACHE[s]


def _make_in_maps(inputs):
    hidden_states = np.asarray(inputs["hidden_states"], dtype=np.float32)
    attention_mask = np.asarray(inputs["attention_mask"], dtype=np.float32)
    Wq = np.asarray(inputs["Wq"], dtype=np.float32)
    Wk = np.asarray(inputs["Wk"], dtype=np.float32)
    Wv = np.asarray(inputs["Wv"], dtype=np.float32)
    bq = np.asarray(inputs["bq"], dtype=np.float32)
    bk = np.asarray(inputs["bk"], dtype=np.float32)
    bv = np.asarray(inputs["bv"], dtype=np.float32)

    in_maps = []
    for core in range(N_CORES):
        b, g = core // 2, core % 2
        js = slice(g * JC, (g + 1) * JC)
        in_maps.append({
            "hidden": np.ascontiguousarray(hidden_states[b]),
            "mask": np.ascontiguousarray(attention_mask[b].reshape(S, 1)),
            "wq": np.ascontiguousarray(Wq[:, js]),
            "wk": np.ascontiguousarray(Wk[:, js]),
            "wv": np.ascontiguousarray(Wv[:, js]),
            "bq": np.ascontiguousarray(bq[js].reshape(JC, 1)),
            "bk": np.ascontiguousarray(bk[js].reshape(JC, 1)),
            "bv": np.ascontiguousarray(bv[js].reshape(1, JC)),
        })
    return in_maps


def kernel(hidden_states, attention_mask, Wq, bq, Wk, bk, Wv, bv):
    from concourse.bass_utils import run_bass_kernel_spmd

    nc = _get_nc()
    in_maps = _make_in_maps(dict(
        hidden_states=hidden_states, attention_mask=attention_mask,
        Wq=Wq, bq=bq, Wk=Wk, bk=bk, Wv=Wv, bv=bv))

    res = run_bass_kernel_spmd(nc, in_maps, core_ids=list(range(N_CORES)))
    out = np.empty((B, S, 16, D), dtype=np.float32)
    for core in range(N_CORES):
        b, g = core // 2, core % 2
        out[b, :, g * 8:(g + 1) * 8, :] = \
            res.results[core]["out"].reshape(S, 8, D)
    return out
